# revision 1
# baseline (speedup 1.0000x reference)
"""Trainium2 Bass kernel for nn_NetFV (NetFV pooling head).

Strategy (pure data parallel over 8 cores, 256 batches each):
  - Host: cast x to bf16 twice: natural [B*M, F] and transposed-augmented
    [B, F+1, 608] (row F = ones -> bias fold; cols 600:608 zero pad so every
    128-wide chunk window is in-bounds). Also pre-fold all per-(f,c) finishing
    constants and rearrange the head weights.
  - Device, per superbatch of 8 batches (40 chunks of 120 rows):
      logits chunk [120,8] = matmul(lhsT=XT[61,128] (FWL), rhs=Waug[61,8])
      softmax: one exp / rowsum / recip / mul over the whole superbatch
      fv chunk: matmul(lhsT=Xgrp[120,128]=(x|x^2|ones|pad) (FWL),
                       rhs=act[120,8]) accumulated over 5 chunks per batch
      -> psum [128(=fv1|fv2|asum|junk), 8] per batch
  - Finishing per 64 batches, f-on-partitions: all elementwise work as
    [60,512] DVE ops with const broadcast APs; partition reductions and
    partition broadcasts via tiny PE matmuls with ones vectors; second
    l2_normalize of fv1 folded as 1/sqrt(C) into the head weights; head as
    16 accumulated [60,64]x[60,18] matmuls.
"""

import math
import sys

for _p in ("/opt/trn_rl_repo", "/opt/pypackages"):
    if _p not in sys.path:
        sys.path.append(_p)

import ml_dtypes
import numpy as np

import concourse.bacc as bacc
import concourse.bass as bass
import concourse.mybir as mybir
import concourse.tile as tile
from concourse.bass_utils import run_bass_kernel_spmd

F, M, C, OUT = 60, 600, 8, 18
B = 2048
NCORES = 8
BL = B // NCORES            # 256 batches per core
SB = 8                      # batches per superbatch
NSB = BL // SB              # 32 superbatches
FGB = 64                    # batches per finishing group
NFG = BL // FGB             # 4 finishing groups
SBPF = FGB // SB            # 8 superbatches per finishing group
CH = 5                      # chunks (of 120 rows) per batch
RP = M // CH                # 120 rows per chunk
XTW = 608                   # padded transposed row length
NG = FGB * C                # 512 finishing columns

BF16 = mybir.dt.bfloat16
F32 = mybir.dt.float32
MULT = mybir.AluOpType.mult
EPS = 1e-12


def _build_nc():
    nc = bacc.Bacc(
        "TRN2", target_bir_lowering=False, debug=False,
        enable_asserts=False, num_devices=NCORES,
    )
    # superbatch-packed layouts: one contiguous run per partition per DMA
    xg = nc.dram_tensor("xg", [NSB, RP, SB * CH * F], BF16,
                        kind="ExternalInput").ap()
    xt = nc.dram_tensor("xt", [NSB, F + 1, SB * XTW], BF16,
                        kind="ExternalInput").ap()
    waug_d = nc.dram_tensor("waug", [F + 1, C], BF16, kind="ExternalInput").ap()
    cst_d = nc.dram_tensor("cst", [128, 5 * C], F32, kind="ExternalInput").ap()
    hds_d = nc.dram_tensor("hds", [F, 2 * C * OUT], F32, kind="ExternalInput").ap()
    y = nc.dram_tensor("y", [BL, OUT], F32, kind="ExternalOutput").ap()

    with tile.TileContext(nc) as tc:
        _emit(tc, y, xg, xt, waug_d, cst_d, hds_d)
    nc.compile()
    return nc


def _emit(tc, y, xg, xt, waug_d, cst_d, hds_d):
    nc = tc.nc
    from contextlib import ExitStack
    ctx = ExitStack()
    with ctx:
        cpool = ctx.enter_context(tc.tile_pool(name="cpool", bufs=1))
        xpool = ctx.enter_context(tc.tile_pool(name="xpool", bufs=3))
        tpool = ctx.enter_context(tc.tile_pool(name="tpool", bufs=3))
        spool = ctx.enter_context(tc.tile_pool(name="spool", bufs=3))
        gpool = ctx.enter_context(tc.tile_pool(name="gpool", bufs=2))
        fpool = ctx.enter_context(tc.tile_pool(name="fpool", bufs=2))
        lpsum = ctx.enter_context(tc.tile_pool(name="lpsum", bufs=2, space="PSUM"))
        fpsum = ctx.enter_context(tc.tile_pool(name="fpsum", bufs=2, space="PSUM"))
        apsum = ctx.enter_context(tc.tile_pool(name="apsum", bufs=1, space="PSUM"))
        bpsum = ctx.enter_context(tc.tile_pool(name="bpsum", bufs=1, space="PSUM"))
        rpsum = ctx.enter_context(tc.tile_pool(name="rpsum", bufs=1, space="PSUM"))
        npsum = ctx.enter_context(tc.tile_pool(name="npsum", bufs=1, space="PSUM"))

        # ---- constants ----
        waug = cpool.tile([F + 1, C], BF16)
        nc.sync.dma_start(out=waug[:], in_=waug_d[:])
        cst = cpool.tile([128, 5 * C], F32)
        nc.sync.dma_start(out=cst[:], in_=cst_d[:])
        hds = cpool.tile([F, 2 * C * OUT], F32)
        nc.sync.dma_start(out=hds[:], in_=hds_d[:])
        k1 = cst[0:F, 0 * C:1 * C]
        w2k1 = cst[0:F, 1 * C:2 * C]
        bco64 = cst[64:64 + F, 2 * C:3 * C]   # used against stage[64:124]
        cco = cst[0:F, 3 * C:4 * C]
        dco = cst[0:F, 4 * C:5 * C]

        ones_r = cpool.tile([1, F], F32)   # lhsT for partition-broadcast
        nc.vector.memset(ones_r[:], 1.0)
        ones_c = cpool.tile([F, 1], F32)   # lhsT for partition-sum
        nc.vector.memset(ones_c[:], 1.0)
        eps1 = cpool.tile([1, 1], F32)     # l2-normalize epsilon
        nc.vector.memset(eps1[:], EPS)
        onem = cpool.tile([RP, 1], BF16)   # ones column: asum matmuls
        nc.vector.memset(onem[:], 1.0)

        def cb(ap):  # broadcast a [F, C] const across FGB batches
            return ap.unsqueeze(1).broadcast_to([F, FGB, C])

        for fg in range(NFG):
            stage = gpool.tile([128, NG], F32)
            asumst = gpool.tile([1, NG], F32)
            for s in range(SBPF):
                sb = fg * SBPF + s
                # ---- load superbatch ----
                xgt = xpool.tile([RP, SB * CH * 128], BF16)
                nc.sync.dma_start(
                    out=xgt.rearrange("p (k q) -> p k q", q=128)[:, :, 0:F],
                    in_=xg[sb].rearrange("p (k f) -> p k f", f=F),
                )
                # x^2 into cols 64:124 (32-aligned partition base after matmul)
                xgv = xgt.rearrange("p (k q) -> p k q", q=128)
                nc.vector.tensor_mul(
                    xgv[:, :, 64:64 + F], xgv[:, :, 0:F], xgv[:, :, 0:F]
                )
                xtt = tpool.tile([F + 1, SB * XTW], BF16)
                nc.sync.dma_start(out=xtt[:], in_=xt[sb])
                # ---- logits ----
                lp = lpsum.tile([128, SB * CH * C], F32)
                for b in range(SB):
                    for c in range(CH):
                        nc.tensor.matmul(
                            lp[:, (b * CH + c) * C:(b * CH + c + 1) * C],
                            xtt[:, b * XTW + c * RP: b * XTW + c * RP + 128],
                            waug[:],
                            start=True, stop=True,
                        )
                # ---- softmax over C ----
                expt = spool.tile([RP, SB * CH * C], F32, tag="expt")
                nc.scalar.activation(
                    expt[:], lp[0:RP, :], mybir.ActivationFunctionType.Exp
                )
                sums = spool.tile([RP, SB * CH], F32, tag="sums")
                nc.vector.reduce_sum(
                    out=sums[:],
                    in_=expt.rearrange("p (k e) -> p k e", e=C),
                    axis=mybir.AxisListType.X,
                )
                rin = spool.tile([RP, SB * CH], F32, tag="rin")
                nc.vector.reciprocal(rin[:], sums[:])
                actt = spool.tile([RP, SB * CH * C], BF16, tag="actt")
                nc.vector.tensor_tensor(
                    out=actt.rearrange("p (k e) -> p k e", e=C),
                    in0=expt.rearrange("p (k e) -> p k e", e=C),
                    in1=rin.unsqueeze(2).broadcast_to([RP, SB * CH, C]),
                    op=MULT,
                )
                # ---- fv accumulation ----
                fp = fpsum.tile([128, SB * C], F32)
                ap_ = apsum.tile([1, SB * C], F32)
                for b in range(SB):
                    for c in range(CH):
                        nc.tensor.matmul(
                            fp[:, b * C:(b + 1) * C],
                            xgt[:, (b * CH + c) * 128:(b * CH + c + 1) * 128],
                            actt[:, (b * CH + c) * C:(b * CH + c + 1) * C],
                            start=(c == 0), stop=(c == CH - 1),
                        )
                        nc.tensor.matmul(
                            ap_[:, b * C:(b + 1) * C],
                            onem[:],
                            actt[:, (b * CH + c) * C:(b * CH + c + 1) * C],
                            start=(c == 0), stop=(c == CH - 1),
                        )
                nc.vector.tensor_copy(
                    stage[:, s * SB * C:(s + 1) * SB * C], fp[:]
                )
                nc.scalar.copy(
                    asumst[:, s * SB * C:(s + 1) * SB * C], ap_[:]
                )

            # ---- finishing for this group of 64 batches ----
            fv1r = stage[0:F, :]
            fv2r = stage[64:64 + F, :]
            asb = bpsum.tile([F, NG], F32)
            nc.tensor.matmul(asb[:], ones_r[:], asumst[:], start=True, stop=True)

            t1 = fpool.tile([F, NG], F32, tag="t1")
            nc.vector.tensor_tensor(out=t1.rearrange("p (g e) -> p g e", e=C),
                                    in0=fv1r.rearrange("p (g e) -> p g e", e=C),
                                    in1=cb(k1), op=MULT)
            m1 = fpool.tile([F, NG], F32, tag="m1")
            nc.vector.tensor_tensor(out=m1.rearrange("p (g e) -> p g e", e=C),
                                    in0=asb.rearrange("p (g e) -> p g e", e=C),
                                    in1=cb(w2k1), op=MULT)
            fv1f = fpool.tile([F, NG], F32, tag="fv1f")
            nc.vector.tensor_sub(fv1f[:], t1[:], m1[:])
            q1 = fpool.tile([F, NG], F32, tag="q1")
            nc.vector.tensor_mul(q1[:], fv1f[:], fv1f[:])
            r1 = rpsum.tile([1, NG], F32, tag="rs")
            nc.tensor.matmul(r1[:], ones_c[:], q1[:], start=True, stop=True)
            sq1 = fpool.tile([1, NG], F32, tag="sq1")
            nc.scalar.activation(sq1[:], r1[:],
                                 mybir.ActivationFunctionType.Sqrt, bias=eps1[:])
            nr1 = fpool.tile([1, NG], F32, tag="nr1")
            nc.vector.reciprocal(nr1[:], sq1[:])
            nb1 = npsum.tile([F, NG], F32, tag="nb")
            nc.tensor.matmul(nb1[:], ones_r[:], nr1[:], start=True, stop=True)
            fv1n = fpool.tile([F, NG], F32, tag="fv1n")
            nc.vector.tensor_mul(fv1n[:], fv1f[:], nb1[:])

            u1 = fpool.tile([F, NG], F32, tag="u1")
            nc.vector.tensor_tensor(out=u1.rearrange("p (g e) -> p g e", e=C),
                                    in0=asb.rearrange("p (g e) -> p g e", e=C),
                                    in1=cb(dco), op=MULT)
            u2 = fpool.tile([F, NG], F32, tag="u2")
            nc.vector.tensor_tensor(out=u2.rearrange("p (g e) -> p g e", e=C),
                                    in0=fv2r.rearrange("p (g e) -> p g e", e=C),
                                    in1=bco64.unsqueeze(1).broadcast_to([F, FGB, C]),
                                    op=MULT)
            u3 = fpool.tile([F, NG], F32, tag="u3")
            nc.vector.tensor_add(u3[:], u1[:], u2[:])
            u4 = fpool.tile([F, NG], F32, tag="u4")
            nc.vector.tensor_tensor(out=u4.rearrange("p (g e) -> p g e", e=C),
                                    in0=fv1r.rearrange("p (g e) -> p g e", e=C),
                                    in1=cb(cco), op=MULT)
            fv2n = fpool.tile([F, NG], F32, tag="fv2n")
            nc.vector.tensor_sub(fv2n[:], u3[:], u4[:])
            q2 = fpool.tile([F, NG], F32, tag="q2")
            nc.vector.tensor_mul(q2[:], fv2n[:], fv2n[:])
            r2 = rpsum.tile([1, NG], F32, tag="rs")
            nc.tensor.matmul(r2[:], ones_c[:], q2[:], start=True, stop=True)
            r2c = fpool.tile([1, FGB], F32, tag="r2c")
            nc.vector.reduce_sum(out=r2c[:],
                                 in_=r2.rearrange("p (g e) -> p g e", e=C),
                                 axis=mybir.AxisListType.X)
            sq2 = fpool.tile([1, FGB], F32, tag="sq2")
            nc.scalar.activation(sq2[:], r2c[:],
                                 mybir.ActivationFunctionType.Sqrt, bias=eps1[:])
            nr2 = fpool.tile([1, FGB], F32, tag="nr2")
            nc.vector.reciprocal(nr2[:], sq2[:])
            nr2e = fpool.tile([1, NG], F32, tag="nr2e")
            nc.vector.tensor_copy(
                nr2e.rearrange("p (g e) -> p g e", e=C),
                nr2.unsqueeze(2).broadcast_to([1, FGB, C]),
            )
            nb2 = npsum.tile([F, NG], F32, tag="nb")
            nc.tensor.matmul(nb2[:], ones_r[:], nr2e[:], start=True, stop=True)
            fv2nn = fpool.tile([F, NG], F32, tag="fv2nn")
            nc.vector.tensor_mul(fv2nn[:], fv2n[:], nb2[:])

            # ---- head ----
            hp = rpsum.tile([FGB, OUT], F32, tag="rs")
            for ci in range(C):
                nc.tensor.matmul(
                    hp[:], fv1n[:, ci::C], hds[:, ci * OUT:(ci + 1) * OUT],
                    start=(ci == 0), stop=False,
                )
            for ci in range(C):
                nc.tensor.matmul(
                    hp[:], fv2nn[:, ci::C],
                    hds[:, (C + ci) * OUT:(C + ci + 1) * OUT],
                    start=False, stop=(ci == C - 1),
                )
            yt = fpool.tile([FGB, OUT], F32, tag="yt")
            nc.scalar.copy(yt[:], hp[:])
            nc.sync.dma_start(out=y[fg * FGB:(fg + 1) * FGB, :], in_=yt[:])


def _host_prep(reshaped_input, cluster_weights, covar_weights, cluster_biases,
               cluster_weights2, hidden1_weights):
    bf = ml_dtypes.bfloat16
    x = np.ascontiguousarray(reshaped_input, dtype=np.float32)
    xb = x.astype(bf)                                   # [B*M, F]
    x3 = xb.reshape(B, M, F)
    xtr = np.zeros((B, F + 1, XTW), dtype=bf)
    xtr[:, :F, :M] = x3.transpose(0, 2, 1)
    xtr[:, F, :M] = bf(1.0)
    # superbatch-packed: xgp[core][sb, p, k*F+f], xtp[core][sb, :, b*XTW+q]
    xgp = (xb.reshape(NCORES, NSB, SB * CH, RP, F)
             .transpose(0, 1, 3, 2, 4)
             .reshape(NCORES, NSB, RP, SB * CH * F))
    xtp = (xtr.reshape(NCORES, NSB, SB, F + 1, XTW)
              .transpose(0, 1, 3, 2, 4)
              .reshape(NCORES, NSB, F + 1, SB * XTW))

    waug = np.concatenate(
        [cluster_weights, cluster_biases[None, :]], axis=0
    ).astype(bf)                                        # [61, 8]

    cw = np.square(covar_weights.astype(np.float64)) + 1e-6       # [F, C]
    w2 = cluster_weights2[0].astype(np.float64)                   # [F, C]
    k1 = 1.0 / cw
    w2k1 = w2 / cw
    bcc = 1.0 / np.square(cw)
    ccc = 2.0 * w2 / np.square(cw)
    dcc = np.square(w2) / np.square(cw) - 1.0
    cst60 = np.concatenate([k1, w2k1, bcc, ccc, dcc], axis=1).astype(np.float32)
    cst = np.zeros((128, 5 * C), dtype=np.float32)
    cst[0:F] = cst60
    cst[64:64 + F] = cst60

    h = hidden1_weights.astype(np.float64)              # [2*C*F, OUT]
    h1 = h[:C * F].reshape(F, C, OUT) / math.sqrt(C)    # fold 2nd l2n of fv1
    h2 = h[C * F:].reshape(F, C, OUT)
    hds = np.concatenate([h1, h2], axis=1).reshape(F, 2 * C * OUT)
    hds = np.ascontiguousarray(hds, dtype=np.float32)

    in_maps = []
    for ci in range(NCORES):
        in_maps.append({
            "xg": np.ascontiguousarray(xgp[ci]),
            "xt": np.ascontiguousarray(xtp[ci]),
            "waug": waug,
            "cst": cst,
            "hds": hds,
        })
    return in_maps


_CACHE = {}


def _get_nc():
    if "nc" not in _CACHE:
        _CACHE["nc"] = _build_nc()
    return _CACHE["nc"]


def kernel(reshaped_input, cluster_weights, covar_weights, cluster_biases,
           cluster_weights2, hidden1_weights, **_kw):
    in_maps = _host_prep(reshaped_input, cluster_weights, covar_weights,
                         cluster_biases, cluster_weights2, hidden1_weights)
    nc = _get_nc()
    res = run_bass_kernel_spmd(nc, in_maps, list(range(NCORES)))
    ys = [res.results[ci]["y"] for ci in range(NCORES)]
    return np.ascontiguousarray(np.concatenate(ys, axis=0), dtype=np.float32)


if __name__ == "__main__":
    rng = np.random.default_rng(0)
    fake = {
        "reshaped_input": rng.standard_normal((B * M, F), dtype=np.float32),
        "cluster_weights": rng.standard_normal((F, C)).astype(np.float32) * 0.13,
        "covar_weights": rng.standard_normal((F, C)).astype(np.float32) * 0.13,
        "cluster_biases": rng.standard_normal((C,)).astype(np.float32) * 0.13,
        "cluster_weights2": rng.standard_normal((1, F, C)).astype(np.float32) * 0.13,
        "hidden1_weights": rng.standard_normal((2 * C * F, OUT)).astype(np.float32) * 0.35,
    }
    out = kernel(**fake)
    print("kernel output", out.shape, out.dtype, np.abs(out).mean())



# revision 13
# speedup vs baseline: 1.9070x; 1.9070x over previous
"""Trainium2 Bass kernel for nn_NetFV (NetFV pooling head).

Strategy (pure data parallel over 8 cores, 256 batches each):
  - DMA is fully contiguous (the baseline's strided xg write generated
    153k 120-byte descriptors = 2.7ms of DMA engine time). x arrives as
    [120, 80x64] blocks of (x|1|0,0,0); GPSIMD spreads them into the even
    64-halves of a [120, 80x128] weight tile, DVE/ACT write squares into
    the odd halves. All junk lanes compute to exact zeros.
  - Logits: block-diagonal pairing. Two chunks' x^T stacked on 122
    partitions, rhs = [122, 16] block-diag (W|b), so one matmul yields
    logits for 240 rows -> 640 matmuls/core instead of 1280.
  - fv: weights = (x|1|0|x^2|1|0) [120, 128] FWL blocks, rhs = act
    [120, 8], accumulated over 5 chunks into a [128, 512] PSUM bank per
    64 batches. The ones column makes PSUM partition 60 the act-sum, so
    no separate asum matmuls (baseline spent 1280 of them).
  - Finishing processes fv1 (partitions 0:60) and fv2 (64:124) halves in
    single [124, 512] DVE ops with per-partition folded constants;
    partition reductions/broadcasts via tiny PE matmuls; head emits
    y^T [18, 64] per group, one output DMA per core.
"""

import math
import sys

for _p in ("/opt/trn_rl_repo", "/opt/pypackages"):
    if _p not in sys.path:
        sys.path.append(_p)

import ml_dtypes
import numpy as np

import concourse.bacc as bacc
import concourse.bass as bass
import concourse.mybir as mybir
import concourse.tile as tile
from concourse.bass_utils import run_bass_kernel_spmd

F, M, C, OUT = 60, 600, 8, 18
B = 2048
NCORES = 8
BL = B // NCORES            # 256 batches per core
BPB = 16                    # batches per block
NBLK = BL // BPB            # 16 blocks
CH = 5                      # chunks (of 120 rows) per batch
RP = M // CH                # 120 rows per chunk
FGB = 64                    # batches per finishing group
NFG = BL // FGB             # 4 groups
BPF = FGB // BPB            # 4 blocks per group
XW = 608                    # per-batch transposed window width
NBX = BPB * CH              # 80 blocks per xg tile
SQ_DVE = 48                 # squares: blocks [0, SQ_DVE) on DVE, rest on ACT

BF16 = mybir.dt.bfloat16
F32 = mybir.dt.float32
MULT = mybir.AluOpType.mult
EPS = 1e-12


def _build_nc():
    nc = bacc.Bacc(
        "TRN2", target_bir_lowering=False, debug=False,
        enable_asserts=False, num_devices=NCORES,
    )
    xg = nc.dram_tensor("xg", [NBLK, RP, NBX * 64], BF16,
                        kind="ExternalInput").ap()
    xt = nc.dram_tensor("xt", [NBLK, 2 * (F + 1), 8 * XW], BF16,
                        kind="ExternalInput").ap()
    waug_d = nc.dram_tensor("waug", [2 * (F + 1), 16], BF16,
                            kind="ExternalInput").ap()
    cst_d = nc.dram_tensor("cst", [128, 3 * C], F32, kind="ExternalInput").ap()
    hds_d = nc.dram_tensor("hds", [124, C * OUT], F32,
                           kind="ExternalInput").ap()
    pew_d = nc.dram_tensor("pew", [128, 124], F32, kind="ExternalInput").ap()
    posm_d = nc.dram_tensor("posm", [128, 3], F32, kind="ExternalInput").ap()
    pw3_d = nc.dram_tensor("pw3", [1, 128], F32, kind="ExternalInput").ap()
    y = nc.dram_tensor("y", [OUT, BL], F32, kind="ExternalOutput").ap()

    with tile.TileContext(nc) as tc:
        _emit(tc, y, xg, xt, waug_d, cst_d, hds_d, pew_d, posm_d, pw3_d)
    nc.compile()
    return nc


def _emit(tc, y, xg, xt, waug_d, cst_d, hds_d, pew_d, posm_d, pw3_d):
    nc = tc.nc
    from contextlib import ExitStack
    ctx = ExitStack()
    with ctx:
        cpool = ctx.enter_context(tc.tile_pool(name="cpool", bufs=1))
        xspool = ctx.enter_context(tc.tile_pool(name="xspool", bufs=3))
        xpool = ctx.enter_context(tc.tile_pool(name="xpool", bufs=2))
        tpool = ctx.enter_context(tc.tile_pool(name="tpool", bufs=3))
        spool = ctx.enter_context(tc.tile_pool(name="spool", bufs=3))
        gpool = ctx.enter_context(tc.tile_pool(name="gpool", bufs=2))
        fpool = ctx.enter_context(tc.tile_pool(name="fpool", bufs=2))
        ypool = ctx.enter_context(tc.tile_pool(name="ypool", bufs=1))
        lpsum = ctx.enter_context(tc.tile_pool(name="lpsum", bufs=3, space="PSUM"))
        fpsum = ctx.enter_context(tc.tile_pool(name="fpsum", bufs=2, space="PSUM"))
        psA = ctx.enter_context(tc.tile_pool(name="psA", bufs=1, space="PSUM"))
        psB = ctx.enter_context(tc.tile_pool(name="psB", bufs=1, space="PSUM"))

        # ---- constants ----
        waug2 = cpool.tile([2 * (F + 1), 16], BF16)
        nc.sync.dma_start(out=waug2[:], in_=waug_d[:])
        cst = cpool.tile([128, 3 * C], F32)
        nc.sync.dma_start(out=cst[:], in_=cst_d[:])
        hds = cpool.tile([124, C * OUT], F32)
        nc.sync.dma_start(out=hds[:], in_=hds_d[:])

        pew = cpool.tile([128, 124], F32)    # asum extract+broadcast weights
        nc.sync.dma_start(out=pew[:], in_=pew_d[:])
        posm = cpool.tile([128, 3], F32)     # half-sum + row-select weights
        nc.sync.dma_start(out=posm[:], in_=posm_d[:])
        pw3 = cpool.tile([1, 128], F32)      # norm-broadcast row weights
        nc.sync.dma_start(out=pw3[:], in_=pw3_d[:])
        eps1 = cpool.tile([1, 1], F32)
        nc.vector.memset(eps1[:], EPS)
        yts = ypool.tile([OUT, BL], F32)

        for fg in range(NFG):
            fp = fpsum.tile([128, FGB * C], F32)
            for b4 in range(BPF):
                blk = fg * BPF + b4
                # ---- load + build fv weight tile ----
                xgs = xspool.tile([RP, NBX * 64], BF16)
                nc.sync.dma_start(out=xgs[:], in_=xg[blk])
                xgt = xpool.tile([RP, NBX * 128], BF16)
                xgv = xgt.rearrange("p (k q) -> p k q", q=128)
                xsv = xgs.rearrange("p (k q) -> p k q", q=64)
                nc.gpsimd.tensor_copy(xgv[:, :, 0:64], xsv[:, :, :])
                nc.vector.tensor_tensor(
                    out=xgv[:, 0:SQ_DVE, 64:128],
                    in0=xsv[:, 0:SQ_DVE, :], in1=xsv[:, 0:SQ_DVE, :], op=MULT,
                )
                nc.scalar.square(
                    xgv[:, SQ_DVE:NBX, 64:128], xsv[:, SQ_DVE:NBX, :]
                )
                xtt = tpool.tile([2 * (F + 1), 8 * XW], BF16)
                nc.sync.dma_start(out=xtt[:], in_=xt[blk])

                for h in range(2):
                    # ---- logits: 20 block-diag pair matmuls ----
                    lp = lpsum.tile([128, 20 * 16], F32)
                    for g in range(4):
                        for c in range(CH):
                            off = (h * 4 + g) * XW + c * RP
                            p = g * CH + c
                            nc.tensor.matmul(
                                lp[:, p * 16:(p + 1) * 16],
                                xtt[:, off:off + 128],
                                waug2[:],
                                start=True, stop=True,
                            )
                    # ---- softmax ----
                    expt = spool.tile([RP, 320], BF16, tag="expt")
                    nc.scalar.activation(
                        expt[:], lp[0:RP, :], mybir.ActivationFunctionType.Exp
                    )
                    sums = spool.tile([RP, 40], F32, tag="sums")
                    nc.vector.reduce_sum(
                        out=sums[:],
                        in_=expt.rearrange("p (k e) -> p k e", e=C),
                        axis=mybir.AxisListType.X,
                    )
                    rin = spool.tile([RP, 40], F32, tag="rin")
                    nc.vector.reciprocal(rin[:], sums[:])
                    actt = spool.tile([RP, 320], BF16, tag="actt")
                    nc.vector.tensor_tensor(
                        out=actt.rearrange("p (k e) -> p k e", e=C),
                        in0=expt.rearrange("p (k e) -> p k e", e=C),
                        in1=rin.unsqueeze(2).broadcast_to([RP, 40, C]),
                        op=MULT,
                    )
                    # ---- fv accumulation ----
                    for g8 in range(8):
                        b16 = h * 8 + g8
                        pc = (b4 * BPB + b16) * C
                        for c in range(CH):
                            acol = ((g8 % 4) * CH + c) * 16 + (g8 // 4) * C
                            nc.tensor.matmul(
                                fp[:, pc:pc + C],
                                xgt[:, (b16 * CH + c) * 128:
                                       (b16 * CH + c + 1) * 128],
                                actt[:, acol:acol + C],
                                start=(c == 0), stop=(c == CH - 1),
                            )

            # ---- finishing for 64 batches ----
            NG = FGB * C
            stage = gpool.tile([128, NG], F32)
            nc.vector.tensor_copy(stage[:], fp[:])
            asb = psA.tile([124, NG], F32, tag="pA")
            nc.tensor.matmul(asb[:], pew[:], stage[:],
                             start=True, stop=True)

            def b3(ap):
                return ap.rearrange("p (g e) -> p g e", e=C)

            def cb(col, pr=124):
                return (cst[0:pr, col * C:(col + 1) * C]
                        .unsqueeze(1).broadcast_to([pr, FGB, C]))

            X1 = fpool.tile([124, NG], F32, tag="X1")
            nc.vector.tensor_tensor(out=b3(X1), in0=b3(stage[0:124, :]),
                                    in1=cb(0), op=MULT)
            X2 = fpool.tile([124, NG], F32, tag="X2")
            nc.vector.tensor_tensor(out=b3(X2), in0=b3(asb), in1=cb(1),
                                    op=MULT)
            X3 = fpool.tile([124, NG], F32, tag="X3")
            nc.vector.tensor_add(X3[:], X1[:], X2[:])
            X5 = fpool.tile([124, NG], F32, tag="X5")
            nc.vector.tensor_tensor(out=b3(X5[64:64 + F, :]),
                                    in0=b3(stage[0:F, :]),
                                    in1=cb(2, F), op=MULT)
            nc.vector.tensor_sub(X3[64:64 + F, :], X3[64:64 + F, :],
                                 X5[64:64 + F, :])
            Q = fpool.tile([124, NG], F32, tag="Q")
            nc.vector.tensor_mul(Q[:], X3[:], X3[:])
            r = psB.tile([2, NG], F32, tag="pB")
            nc.tensor.matmul(r[:], posm[0:124, 0:2], Q[:],
                             start=True, stop=True)
            rb = fpool.tile([2, NG], F32, tag="rb")
            nc.vector.tensor_copy(rb[:], r[:])
            # fv1 norms: per (batch, cluster) over F
            sqA = fpool.tile([1, NG], F32, tag="sqA")
            nc.scalar.activation(sqA[:], rb[0:1, :],
                                 mybir.ActivationFunctionType.Sqrt,
                                 bias=eps1[:])
            nrA = fpool.tile([1, NG], F32, tag="nrA")
            nc.vector.reciprocal(nrA[:], sqA[:])
            # fv2 norm: summed over clusters per batch (row 1 of rb)
            rc2 = fpool.tile([2, FGB], F32, tag="rc2")
            nc.vector.reduce_sum(
                out=rc2[:],
                in_=rb.rearrange("p (g e) -> p g e", e=C),
                axis=mybir.AxisListType.X,
            )
            rx2 = psA.tile([1, FGB], F32, tag="pA")
            nc.tensor.matmul(rx2[:], posm[0:2, 2:3], rc2[:],
                             start=True, stop=True)
            rxs = fpool.tile([1, FGB], F32, tag="rxs")
            nc.vector.tensor_copy(rxs[:], rx2[:])
            sqB = fpool.tile([1, FGB], F32, tag="sqB")
            nc.scalar.activation(sqB[:], rxs[:],
                                 mybir.ActivationFunctionType.Sqrt,
                                 bias=eps1[:])
            nrB = fpool.tile([1, FGB], F32, tag="nrB")
            nc.vector.reciprocal(nrB[:], sqB[:])
            nrBe = fpool.tile([1, NG], F32, tag="nrBe")
            nc.vector.tensor_copy(
                nrBe.rearrange("p (g e) -> p g e", e=C),
                nrB.unsqueeze(2).broadcast_to([1, FGB, C]),
            )
            nb = psA.tile([124, NG], F32, tag="pA")
            nc.tensor.matmul(nb[0:64, :], pw3[0:1, 0:64], nrA[:],
                             start=True, stop=True)
            nc.tensor.matmul(nb[64:124, :], pw3[0:1, 64:124], nrBe[:],
                             start=True, stop=True)
            fvn = fpool.tile([124, NG], F32, tag="fvn")
            nc.vector.tensor_mul(fvn[:], X3[:], nb[:])
            hp = psB.tile([OUT, FGB], F32, tag="pB")
            fvv = fvn.rearrange("p (g e) -> p g e", e=C)
            for ci in range(C):
                nc.tensor.matmul(
                    hp[:], hds[:, ci * OUT:(ci + 1) * OUT], fvv[:, :, ci],
                    start=(ci == 0), stop=(ci == C - 1),
                )
            nc.scalar.copy(yts[:, fg * FGB:(fg + 1) * FGB], hp[:])
        nc.sync.dma_start(out=y[:], in_=yts[:])


def _host_prep(reshaped_input, cluster_weights, covar_weights, cluster_biases,
               cluster_weights2, hidden1_weights):
    bf = ml_dtypes.bfloat16
    xb = np.ascontiguousarray(reshaped_input, dtype=np.float32).astype(bf)
    xb = xb.reshape(B, M, F)
    # xg: [cores, NBLK, 120, 80*64] of (x | 1 | 0 0 0) blocks
    x6 = xb.reshape(NCORES, NBLK, BPB, CH, RP, F)
    xgp = np.zeros((NCORES, NBLK, RP, BPB, CH, 64), dtype=bf)
    xgp[..., :F] = x6.transpose(0, 1, 4, 2, 3, 5)
    xgp[..., F] = bf(1.0)
    xgp = xgp.reshape(NCORES, NBLK, RP, NBX * 64)
    # xt: [cores, NBLK, 122, 8*608]; partitions 0:61 = batches g%8<4 of the
    # block (x^T rows + ones row), 61:122 = batches g%8>=4
    xtr = np.zeros((B, F + 1, XW), dtype=bf)
    xtr[:, :F, :M] = xb.transpose(0, 2, 1)
    xtr[:, F, :M] = bf(1.0)
    x7 = xtr.reshape(NCORES, NBLK, 2, 2, 4, F + 1, XW)
    xtp = (x7.transpose(0, 1, 3, 5, 2, 4, 6)
             .reshape(NCORES, NBLK, 2 * (F + 1), 8 * XW))

    waug = np.concatenate(
        [cluster_weights, cluster_biases[None, :]], axis=0
    ).astype(bf)                                        # [61, 8]
    waug2 = np.zeros((2 * (F + 1), 16), dtype=bf)
    waug2[:F + 1, :C] = waug
    waug2[F + 1:, C:] = waug

    cw = np.square(covar_weights.astype(np.float64)) + 1e-6       # [F, C]
    w2 = cluster_weights2[0].astype(np.float64)                   # [F, C]
    cstA = np.zeros((128, C))
    cstA[0:F] = 1.0 / cw
    cstA[64:64 + F] = 1.0 / np.square(cw)
    cstB = np.zeros((128, C))
    cstB[0:F] = -w2 / cw
    cstB[64:64 + F] = np.square(w2) / np.square(cw) - 1.0
    cstC = np.zeros((128, C))
    cstC[0:F] = 2.0 * w2 / np.square(cw)
    cst = np.concatenate([cstA, cstB, cstC], axis=1).astype(np.float32)

    h = hidden1_weights.astype(np.float64)              # [2*C*F, OUT]
    h1 = h[:C * F].reshape(F, C, OUT) / math.sqrt(C)    # fold 2nd l2n of fv1
    h2 = h[C * F:].reshape(F, C, OUT)
    hds = np.zeros((124, C * OUT))
    hds[0:F] = h1.reshape(F, C * OUT)
    hds[64:64 + F] = h2.reshape(F, C * OUT)
    hds = hds.astype(np.float32)

    pew = np.zeros((128, 124), dtype=np.float32)
    pew[60, :] = 1.0
    posm = np.zeros((128, 3), dtype=np.float32)
    posm[0:F, 0] = 1.0
    posm[64:64 + F, 1] = 1.0
    posm[1, 2] = 1.0
    pw3 = np.zeros((1, 128), dtype=np.float32)
    pw3[0, 0:F] = 1.0
    pw3[0, 64:64 + F] = 1.0

    in_maps = []
    for ci in range(NCORES):
        in_maps.append({
            "xg": np.ascontiguousarray(xgp[ci]),
            "xt": np.ascontiguousarray(xtp[ci]),
            "waug": waug2,
            "cst": cst,
            "hds": hds,
            "pew": pew,
            "posm": posm,
            "pw3": pw3,
        })
    return in_maps


_CACHE = {}


def _get_nc():
    if "nc" not in _CACHE:
        _CACHE["nc"] = _build_nc()
    return _CACHE["nc"]


def kernel(reshaped_input, cluster_weights, covar_weights, cluster_biases,
           cluster_weights2, hidden1_weights, **_kw):
    in_maps = _host_prep(reshaped_input, cluster_weights, covar_weights,
                         cluster_biases, cluster_weights2, hidden1_weights)
    nc = _get_nc()
    res = run_bass_kernel_spmd(nc, in_maps, list(range(NCORES)))
    ys = [res.results[ci]["y"].T for ci in range(NCORES)]
    return np.ascontiguousarray(np.concatenate(ys, axis=0), dtype=np.float32)


if __name__ == "__main__":
    rng = np.random.default_rng(0)
    fake = {
        "reshaped_input": rng.standard_normal((B * M, F), dtype=np.float32),
        "cluster_weights": rng.standard_normal((F, C)).astype(np.float32) * 0.13,
        "covar_weights": rng.standard_normal((F, C)).astype(np.float32) * 0.13,
        "cluster_biases": rng.standard_normal((C,)).astype(np.float32) * 0.13,
        "cluster_weights2": rng.standard_normal((1, F, C)).astype(np.float32) * 0.13,
        "hidden1_weights": rng.standard_normal((2 * C * F, OUT)).astype(np.float32) * 0.35,
    }
    out = kernel(**fake)
    print("kernel output", out.shape, out.dtype, np.abs(out).mean())
